# revision 1
# baseline (speedup 1.0000x reference)
"""HELMo encoder (bi-GRU over 3 steps + MHA + classifier) on 8 trn2 cores.

Data-parallel over batch (8192 -> 8 x 1024). Per core, one Bass/Tile kernel:
  A) fused GRU: input and hidden projections accumulate into shared PSUM
     (k = [x; h_prev] against W_cat = [W_ih.T; W_hh.T]), gates on ACT/DVE,
     feature-major layout (features on partitions, batch on free dim).
  B) Q/K/V projections emitted batch-major directly by using hs chunks as the
     matmul stationary operand (out[b, d_out] = hs[d_in, b].T @ W.T[d_in, d_out]).
  C) attention combine on DVE: per-head segment-reduce logits, softmax,
     then ctx_sum = sum_tk (sum_tq w[h,tq,tk]) * V[tk]  (Wo folded over t).
  D) att = ctx_sum @ Wo.T back in feature-major via PE transposes of ctx_sum.
  E) o = att.T @ W_out.T + b_out, softmax over 7 classes.

All big matmuls run in float32r (~1.3e-4 rel err, full PE rate).
"""

import sys

sys.path.insert(0, "/opt/trn_rl_repo")

import numpy as np

import concourse.bacc as bacc
import concourse.bass as bass
import concourse.mybir as mybir
import concourse.tile as tile
from concourse.masks import make_identity

dt = mybir.dt
AF = mybir.ActivationFunctionType
AX = mybir.AxisListType

N_CORES = 8
B = 8192
B_LOC = B // N_CORES          # 1024
I = 1024
H = 1024
D = 2 * H                     # 2048
NH = 16
HD = 128
S = 3
C = 7
P = 128
HJT = H // P                  # 8 jtiles per gate
KC_D = D // P                 # 16

_CACHE = {}


def _r3(ap, pat, **kw):
    return ap.rearrange(pat, **kw)


def build_nc(phases="abcde", reps=1):
    nc = bacc.Bacc("TRN2", target_bir_lowering=False, debug=False,
                   num_devices=N_CORES, dynamic_dma_scratch_size=8192)

    f32, f32r = dt.float32, dt.float32r
    xt = nc.dram_tensor("xt", [S, I, B_LOC], f32r, kind="ExternalInput")
    wcat = {d: nc.dram_tensor(f"wcat_{d}", [2 * H, 3 * H], f32r, kind="ExternalInput")
            for d in ("f", "b")}
    wq = nc.dram_tensor("wq", [D, D], dt.bfloat16, kind="ExternalInput")
    wk = nc.dram_tensor("wk", [D, D], dt.bfloat16, kind="ExternalInput")
    wv = nc.dram_tensor("wv", [D, D], f32r, kind="ExternalInput")
    wo = nc.dram_tensor("wo", [D, D], f32r, kind="ExternalInput")
    wout = nc.dram_tensor("wout", [D, C], f32, kind="ExternalInput")
    brz = {d: nc.dram_tensor(f"brz_{d}", [2 * H, 1], f32, kind="ExternalInput")
           for d in ("f", "b")}
    negbz = {d: nc.dram_tensor(f"negbz_{d}", [H, 1], f32, kind="ExternalInput")
             for d in ("f", "b")}
    bnih = {d: nc.dram_tensor(f"bnih_{d}", [H, 1], f32, kind="ExternalInput")
            for d in ("f", "b")}
    bnhh = {d: nc.dram_tensor(f"bnhh_{d}", [H, 1], f32, kind="ExternalInput")
            for d in ("f", "b")}
    bout = nc.dram_tensor("bout", [1, C], f32, kind="ExternalInput")
    o_out = nc.dram_tensor("o_out", [B_LOC, C], f32, kind="ExternalOutput")
    sm_out = nc.dram_tensor("sm_out", [B_LOC, C], f32, kind="ExternalOutput")

    with tile.TileContext(nc) as tc:
      for _rep in range(reps):
        with tc.tile_pool(name="dram", bufs=1, space="DRAM") as dram:
            hs = dram.tile([S, D, B_LOC], f32r)
            hs_bf = dram.tile([S, D, B_LOC], dt.bfloat16)
            qs = dram.tile([S, B_LOC, D], dt.bfloat16)
            ks = dram.tile([S, B_LOC, D], dt.bfloat16)
            vs = dram.tile([S, B_LOC, D], f32)
            att_d = dram.tile([D, B_LOC], f32)

            # ---------------- Phase A: GRU ----------------
            if "a" in phases:
              with (tc.tile_pool(name="ga_const", bufs=1) as cpool,
                  tc.tile_pool(name="ga_x", bufs=3) as xpool,
                  tc.tile_pool(name="ga_h", bufs=3) as hpool,
                  tc.tile_pool(name="ga_w", bufs=2) as wpool,
                  tc.tile_pool(name="ga_g", bufs=2) as gpool,
                  tc.tile_pool(name="ga_t", bufs=3) as tpool,
                  tc.tile_pool(name="ga_ps", bufs=2, space="PSUM") as pps):
                bias = {}
                for d in ("f", "b"):
                    t_brz = cpool.tile([P, 2 * HJT, 1], f32, tag=f"brz{d}")
                    nc.sync.dma_start(t_brz[:], _r3(brz[d][:], "(c k) o -> k c o", k=P))
                    t_nbz = cpool.tile([P, HJT, 1], f32, tag=f"nbz{d}")
                    nc.sync.dma_start(t_nbz[:], _r3(negbz[d][:], "(c k) o -> k c o", k=P))
                    t_bni = cpool.tile([P, HJT, 1], f32, tag=f"bni{d}")
                    nc.sync.dma_start(t_bni[:], _r3(bnih[d][:], "(c k) o -> k c o", k=P))
                    t_bnh = cpool.tile([P, HJT, 1], f32, tag=f"bnh{d}")
                    nc.sync.dma_start(t_bnh[:], _r3(bnhh[d][:], "(c k) o -> k c o", k=P))
                    bias[d] = (t_brz, t_nbz, t_bni, t_bnh)

                order = [(0, "f", 0), (0, "b", 2), (1, "f", 1),
                         (1, "b", 1), (2, "f", 2), (2, "b", 0)]
                h_cur = {"f": None, "b": None}
                for step, d, t in order:
                    t_brz, t_nbz, t_bni, t_bnh = bias[d]
                    first = step == 0
                    x_halves = []
                    for xh in range(2):
                        xv = xpool.tile([P, HJT // 2, B_LOC], f32r, tag="x",
                                        name=f"x_{step}_{d}_{xh}")
                        nc.sync.dma_start(
                            xv[:],
                            _r3(xt[t][xh * (I // 2):(xh + 1) * (I // 2), :],
                                "(c k) b -> k c b", k=P))
                        x_halves.append(xv)

                    def x_chunk(c):
                        return x_halves[c // (HJT // 2)][:, c % (HJT // 2), :]
                    h_prev = h_cur[d]
                    h_new = hpool.tile([P, HJT, B_LOC], f32r, tag="h")
                    for j in range(HJT):
                        # host pre-permutes wcat columns: per j the r/z/n gate
                        # columns are adjacent -> one contiguous 384-col DMA
                        nkc = HJT if first else 2 * HJT
                        wj = wpool.tile([P, nkc, 3 * P], f32r, tag="wj",
                                        name=f"wj_{step}_{d}_{j}")
                        nc.sync.dma_start(
                            wj[:],
                            _r3(wcat[d][:nkc * P, j * 3 * P:(j + 1) * 3 * P],
                                "(c k) m -> k c m", k=P))
                        wslice = {"wr": wj[:, :, 0:P], "wz": wj[:, :, P:2 * P],
                                  "wn": wj[:, :, 2 * P:3 * P]}
                        for bt in range(2):
                            bs = slice(bt * 512, (bt + 1) * 512)
                            nk = HJT if first else 2 * HJT

                            def mm_acc(ptile, ws):
                                for c in range(nk):
                                    rhs = (x_chunk(c)[:, bs] if c < HJT
                                           else h_prev[:, c - HJT, bs])
                                    nc.tensor.matmul(ptile[:], ws[:, c, :], rhs,
                                                     start=(c == 0),
                                                     stop=(c == nk - 1))

                            pr = pps.tile([P, 512], f32, tag="pr")
                            mm_acc(pr, wslice["wr"])
                            pz = pps.tile([P, 512], f32, tag="pz")
                            mm_acc(pz, wslice["wz"])
                            pgi = pps.tile([P, 512], f32, tag="pgi")
                            for c in range(HJT):
                                nc.tensor.matmul(pgi[:], wslice["wn"][:, c, :],
                                                 x_chunk(c)[:, bs],
                                                 start=(c == 0), stop=(c == HJT - 1))
                            r_sb = gpool.tile([P, 512], f32, tag="r")
                            nc.scalar.activation(r_sb[:], pr[:], AF.Sigmoid,
                                                 bias=t_brz[:, j, :])
                            n_sb = gpool.tile([P, 512], f32, tag="n")
                            if first:
                                zc = gpool.tile([P, 512], f32, tag="z")
                                nc.scalar.activation(zc[:], pz[:], AF.Sigmoid,
                                                     bias=t_nbz[:, j, :], scale=-1.0)
                                nc.scalar.activation(n_sb[:], pgi[:], AF.Tanh,
                                                     bias=t_bni[:, j, :])
                                nc.vector.tensor_mul(h_new[:, j, bs], zc[:], n_sb[:])
                            else:
                                z_sb = gpool.tile([P, 512], f32, tag="z")
                                nc.scalar.activation(z_sb[:], pz[:], AF.Sigmoid,
                                                     bias=t_brz[:, HJT + j, :])
                                pgh = pps.tile([P, 512], f32, tag="pgh")
                                for c in range(HJT, 2 * HJT):
                                    nc.tensor.matmul(pgh[:], wslice["wn"][:, c, :],
                                                     h_prev[:, c - HJT, bs],
                                                     start=(c == HJT),
                                                     stop=(c == 2 * HJT - 1))
                                t1 = tpool.tile([P, 512], f32, tag="tmp")
                                nc.vector.tensor_scalar_add(t1[:], pgh[:],
                                                            t_bnh[:, j, :])
                                t2 = tpool.tile([P, 512], f32, tag="tmp")
                                nc.vector.tensor_mul(t2[:], r_sb[:], t1[:])
                                t3 = tpool.tile([P, 512], f32, tag="tmp")
                                nc.vector.tensor_add(t3[:], pgi[:], t2[:])
                                nc.scalar.activation(n_sb[:], t3[:], AF.Tanh,
                                                     bias=t_bni[:, j, :])
                                t4 = tpool.tile([P, 512], f32, tag="tmp")
                                nc.vector.tensor_sub(t4[:], h_prev[:, j, bs], n_sb[:])
                                t5 = tpool.tile([P, 512], f32, tag="tmp")
                                nc.vector.tensor_mul(t5[:], z_sb[:], t4[:])
                                nc.vector.tensor_add(h_new[:, j, bs], t5[:], n_sb[:])
                            row = (0 if d == "f" else H) + j * P
                            nc.sync.dma_start(hs[t, row:row + P, bs],
                                              h_new[:, j, bs])
                            hb = tpool.tile([P, 512], dt.bfloat16, tag="hbf",
                                            name=f"hbf_{step}_{d}_{j}_{bt}")
                            nc.vector.tensor_copy(hb[:], h_new[:, j, bs])
                            nc.sync.dma_start(hs_bf[t, row:row + P, bs], hb[:])
                    h_cur[d] = h_new

            # ---------------- Phase B: Q/K/V projections ----------------
            if "b" in phases:
              for wsrc, dst, odt, mmdt in (
                      (wq, qs, dt.bfloat16, dt.bfloat16),
                      (wk, ks, dt.bfloat16, dt.bfloat16),
                      (wv, vs, f32, f32r)):
                hsrc = hs_bf if mmdt == dt.bfloat16 else hs
                with (tc.tile_pool(name="gb_w", bufs=1) as wbpool,
                      tc.tile_pool(name="gb_s", bufs=4) as spool,
                      tc.tile_pool(name="gb_o", bufs=3) as opool,
                      tc.tile_pool(name="gb_ps", bufs=8, space="PSUM") as pps):
                    wt = wbpool.tile([P, KC_D, D], mmdt, tag="wproj",
                                     name=f"wproj_{wsrc.name}")
                    nc.sync.dma_start(wt[:], _r3(wsrc[:], "(c k) n -> k c n", k=P))
                    for t in range(S):
                        for btile in range(HJT):
                            hst = spool.tile([P, KC_D, P], mmdt, tag="hst",
                                             name=f"hst_{wsrc.name}_{t}_{btile}")
                            nc.sync.dma_start(
                                hst[:],
                                _r3(hsrc[t][:, btile * P:(btile + 1) * P],
                                    "(c k) b -> k c b", k=P))
                            osb = opool.tile([P, D], odt, tag="qkvout",
                                             name=f"qkvout_{t}_{btile}")
                            for do_ in range(4):
                                po = pps.tile([P, 512], f32, tag="pqkv",
                                              name=f"pqkv_{t}_{btile}_{do_}")
                                for c in range(KC_D):
                                    nc.tensor.matmul(
                                        po[:], hst[:, c, :],
                                        wt[:, c, do_ * 512:(do_ + 1) * 512],
                                        start=(c == 0), stop=(c == KC_D - 1))
                                nc.scalar.copy(osb[:, do_ * 512:(do_ + 1) * 512],
                                               po[:])
                            nc.sync.dma_start(
                                dst[t][btile * P:(btile + 1) * P, :], osb[:])

            # ---------------- Phase C: attention combine ----------------
            if "c" in phases:
              with (tc.tile_pool(name="gc_inqk", bufs=1) as inqk_pool,
                  tc.tile_pool(name="gc_inv", bufs=2) as inv_pool,
                  tc.tile_pool(name="gc_w", bufs=2) as wkpool,
                  tc.tile_pool(name="gc_t", bufs=1) as tmpool,
                  tc.tile_pool(name="gc_c", bufs=2) as ctxpool,
                  tc.tile_pool(name="gc_m", bufs=1) as cm_pool,
                  tc.tile_pool(name="gc_wo", bufs=2) as wopool,
                  tc.tile_pool(name="gc_1", bufs=1) as one_pool,
                  tc.tile_pool(name="gc_ps", bufs=2, space="PSUM") as pps):
                ident = one_pool.tile([P, P], f32, tag="ident")
                make_identity(nc, ident[:])
                ctxm = cm_pool.tile([P, KC_D, B_LOC], f32r, tag="ctxm")
                for btile in range(HJT):
                    bsl = slice(btile * P, (btile + 1) * P)
                    qt, kt, vt = [], [], []
                    for t in range(S):
                        for src_, lst, nm, pool_ in (
                                (qs, qt, "q", inqk_pool), (ks, kt, "k", inqk_pool),
                                (vs, vt, "v", inv_pool)):
                            tl = pool_.tile([P, D],
                                            dt.bfloat16 if nm in ("q", "k") else f32,
                                            tag=f"{nm}{t}",
                                            name=f"{nm}{t}_{btile}")
                            nc.sync.dma_start(tl[:], src_[t][bsl, :])
                            lst.append(tl)
                    L = wkpool.tile([P, NH, S, S], f32, tag="L")
                    prod_tag = 0
                    for tq in range(S):
                        for tk in range(S):
                            pr_ = wkpool.tile([P, D], dt.bfloat16, tag="prod",
                                              name=f"prod_{btile}_{tq}_{tk}")
                            nc.vector.tensor_mul(pr_[:], qt[tq][:], kt[tk][:])
                            nc.vector.reduce_sum(
                                L[:, :, tq, tk],
                                _r3(pr_[:], "p (h e) -> p h e", h=NH), axis=AX.X)
                            prod_tag += 1
                    # logits are O(1e-3): exp cannot overflow, skip max-sub
                    E2 = wkpool.tile([P, NH, S, S], f32, tag="E2")
                    nc.scalar.activation(E2[:], L[:], AF.Exp)
                    Ssum = wkpool.tile([P, NH, S], f32, tag="Ssum")
                    nc.vector.reduce_sum(Ssum[:], E2[:], axis=AX.X)
                    Rs = wkpool.tile([P, NH, S], f32, tag="Rs")
                    nc.vector.reciprocal(Rs[:], Ssum[:])
                    Wn = wkpool.tile([P, NH, S, S], f32, tag="Wn")
                    nc.vector.tensor_mul(Wn[:], E2[:],
                                         Rs[:, :, :, None].broadcast_to([P, NH, S, S]))
                    wsum = wkpool.tile([P, NH, S], f32, tag="wsum")
                    nc.vector.reduce_sum(wsum[:], _r3(Wn[:], "p h q k -> p h k q"),
                                         axis=AX.X)
                    ctx = ctxpool.tile([P, D], f32, tag="ctx")
                    tm0 = tmpool.tile([P, D], f32, tag="ctmp0")
                    nc.vector.tensor_mul(
                        _r3(tm0[:], "p (h e) -> p h e", h=NH),
                        _r3(vt[0][:], "p (h e) -> p h e", h=NH),
                        wsum[:, :, 0][:, :, None].broadcast_to([P, NH, HD]))
                    tm1 = tmpool.tile([P, D], f32, tag="ctmp1")
                    nc.vector.tensor_mul(
                        _r3(tm1[:], "p (h e) -> p h e", h=NH),
                        _r3(vt[1][:], "p (h e) -> p h e", h=NH),
                        wsum[:, :, 1][:, :, None].broadcast_to([P, NH, HD]))
                    nc.vector.tensor_add(tm0[:], tm0[:], tm1[:])
                    nc.vector.tensor_mul(
                        _r3(tm1[:], "p (h e) -> p h e", h=NH),
                        _r3(vt[2][:], "p (h e) -> p h e", h=NH),
                        wsum[:, :, 2][:, :, None].broadcast_to([P, NH, HD]))
                    nc.vector.tensor_add(ctx[:], tm0[:], tm1[:])
                    for c in range(KC_D):
                        pt = pps.tile([P, P], f32, tag="ptr")
                        nc.tensor.transpose(pt[:], ctx[:, c * P:(c + 1) * P],
                                            ident[:])
                        nc.vector.tensor_copy(ctxm[:, c, bsl], pt[:])
                    # after each half of the btiles, run the Wo half-pass on PE
                    # so it overlaps the DVE combine of the remaining btiles
                    if btile in (3, 7):
                        bt = btile // 4
                        bs = slice(bt * 512, (bt + 1) * 512)
                        for jt in range(KC_D):
                            wos = wopool.tile([P, KC_D, P], f32r, tag="wos",
                                              name=f"wos_{bt}_{jt}")
                            nc.sync.dma_start(
                                wos[:],
                                _r3(wo[:, jt * P:(jt + 1) * P],
                                    "(c k) m -> k c m", k=P))
                            pw = pps.tile([P, 512], f32, tag="pwo",
                                          name=f"pwo_{bt}_{jt}")
                            for c in range(KC_D):
                                nc.tensor.matmul(pw[:], wos[:, c, :],
                                                 ctxm[:, c, bs],
                                                 start=(c == 0),
                                                 stop=(c == KC_D - 1))
                            asb = ctxpool.tile([P, 512], f32, tag="asb",
                                               name=f"asb_{bt}_{jt}")
                            nc.vector.tensor_copy(asb[:], pw[:])
                            nc.sync.dma_start(att_d[jt * P:(jt + 1) * P, bs],
                                              asb[:])

            # ---------------- Phase E: classifier + softmax ----------------
            if "d" in phases:
                with (tc.tile_pool(name="ge", bufs=2) as epool,
                      tc.tile_pool(name="ge1", bufs=1) as e1pool,
                      tc.tile_pool(name="ge_ps", bufs=2, space="PSUM") as pps2):
                    wout_sb = e1pool.tile([P, KC_D, C], f32, tag="wout")
                    nc.sync.dma_start(wout_sb[:], _r3(wout[:], "(c k) n -> k c n", k=P))
                    bout_sb = e1pool.tile([P, C], f32, tag="bout")
                    nc.sync.dma_start(bout_sb[:], bout[:].to_broadcast([P, C]))
                    for btile in range(HJT):
                        bsl = slice(btile * P, (btile + 1) * P)
                        attt = epool.tile([P, KC_D, P], f32, tag="attt",
                                          name=f"attt_{btile}")
                        nc.sync.dma_start(attt[:], _r3(att_d[:, bsl],
                                                       "(c k) b -> k c b", k=P))
                        pf = pps2.tile([P, C], f32, tag="pf")
                        for c in range(KC_D):
                            nc.tensor.matmul(pf[:], attt[:, c, :],
                                             wout_sb[:, c, :],
                                             start=(c == 0), stop=(c == KC_D - 1))
                        o_sb = epool.tile([P, C], f32, tag="osb")
                        nc.vector.tensor_add(o_sb[:], pf[:], bout_sb[:])
                        nc.sync.dma_start(o_out[bsl, :], o_sb[:])
                        mx = epool.tile([P, 1], f32, tag="mx")
                        nc.vector.reduce_max(mx[:], o_sb[:], axis=AX.X)
                        nmx = epool.tile([P, 1], f32, tag="nmx")
                        nc.vector.tensor_scalar_mul(nmx[:], mx[:], -1.0)
                        esb = epool.tile([P, C], f32, tag="esb")
                        nc.scalar.activation(esb[:], o_sb[:], AF.Exp, bias=nmx[:])
                        ssb = epool.tile([P, 1], f32, tag="ssb")
                        nc.vector.reduce_sum(ssb[:], esb[:], axis=AX.X)
                        rsb = epool.tile([P, 1], f32, tag="rsb")
                        nc.vector.reciprocal(rsb[:], ssb[:])
                        smsb = epool.tile([P, C], f32, tag="smsb")
                        nc.vector.tensor_mul(smsb[:], esb[:],
                                             rsb[:].broadcast_to([P, C]))
                        nc.sync.dma_start(sm_out[bsl, :], smsb[:])

    nc.compile()
    return nc


def _prep_inputs(inputs):
    f32 = np.float32
    xs = np.stack([np.asarray(inputs["x1"], f32), np.asarray(inputs["x2"], f32),
                   np.asarray(inputs["x3"], f32)])  # (3, B, I)
    shared = {}
    for d in ("f", "b"):
        wih = np.asarray(inputs[f"W_ih_{d}"], f32)
        whh = np.asarray(inputs[f"W_hh_{d}"], f32)
        bih = np.asarray(inputs[f"b_ih_{d}"], f32)
        bhh = np.asarray(inputs[f"b_hh_{d}"], f32)
        wc = np.concatenate([wih.T, whh.T], axis=0)  # (2I, 3H)
        cols = []
        for j in range(HJT):
            for g in range(3):
                cols.append(wc[:, (g * H + j * P):(g * H + (j + 1) * P)])
        shared[f"wcat_{d}"] = np.ascontiguousarray(np.concatenate(cols, axis=1))
        bsum = bih + bhh
        shared[f"brz_{d}"] = np.ascontiguousarray(bsum[:2 * H, None])
        shared[f"negbz_{d}"] = np.ascontiguousarray(-bsum[H:2 * H, None])
        shared[f"bnih_{d}"] = np.ascontiguousarray(bih[2 * H:, None])
        shared[f"bnhh_{d}"] = np.ascontiguousarray(bhh[2 * H:, None])
    import ml_dtypes
    shared["wq"] = np.ascontiguousarray(
        (np.asarray(inputs["Wq"], f32).T * (HD ** -0.5)).astype(ml_dtypes.bfloat16))
    shared["wk"] = np.ascontiguousarray(
        np.asarray(inputs["Wk"], f32).T.astype(ml_dtypes.bfloat16))
    shared["wv"] = np.ascontiguousarray(np.asarray(inputs["Wv"], f32).T)
    shared["wo"] = np.ascontiguousarray(np.asarray(inputs["Wo"], f32).T)
    shared["wout"] = np.ascontiguousarray(np.asarray(inputs["W_out"], f32).T)
    shared["bout"] = np.ascontiguousarray(np.asarray(inputs["b_out"], f32)[None, :])

    in_maps = []
    for c in range(N_CORES):
        rows = slice(c * B_LOC, (c + 1) * B_LOC)
        m = dict(shared)
        m["xt"] = np.ascontiguousarray(xs[:, rows, :].transpose(0, 2, 1))
        in_maps.append(m)
    return in_maps


def _get_nc():
    if "nc" not in _CACHE:
        _CACHE["nc"] = build_nc()
    return _CACHE["nc"]


def kernel(**inputs):
    from concourse.bass_utils import run_bass_kernel_spmd

    nc = _get_nc()
    in_maps = _prep_inputs(inputs)
    res = run_bass_kernel_spmd(nc, in_maps, core_ids=list(range(N_CORES)))
    o = np.concatenate([res.results[c]["o_out"] for c in range(N_CORES)], axis=0)
    sm = np.concatenate([res.results[c]["sm_out"] for c in range(N_CORES)], axis=0)
    return o, sm



# revision 29
# speedup vs baseline: 25.9170x; 25.9170x over previous
"""HELMo encoder (bi-GRU over 3 steps + MHA + classifier) on 8 trn2 cores.

Data-parallel over batch (8192 -> 8 x 1024). Per core, one Bass/Tile kernel,
bf16 matmuls (fp8e4 DoubleRow for Q/K), feature-major end to end:

  A) fused GRU: k = [x; h_prev] against W_cat = [W_ih.T; W_hh.T] accumulated
     in PSUM, gates on ACT/DVE. h lives in the SBUF-resident hs tile
     [128, S, 16, B_LOC] (bf16) which doubles as the attention input.
  B1) V projections for all heads feature-major from SBUF hs (weights
     streamed); V staged to DRAM per head. hs also converted to an fp8 copy.
     hs (bf16) is then freed.
  B2) per-head attention: Q/K head slices from fp8 hs via DoubleRow matmuls
     (4x PE rate; logits are O(1e-3) softmax inputs, fp8 noise harmless).
     Logits = per-head partition sums of q*k via one-hot matmuls;
     softmax-over-tk / sum-over-tq / partition-broadcast are tiny one-hot
     PE matmuls; ctx accumulates into SBUF [128, 16, B_LOC] (bf16).
  C) att = Wo.T @ ctx feature-major (weights streamed, 512-wide psum).
  D) o = att.T @ W_out.T + b_out batch-major (att chunks stationary),
     softmax over 7 classes.
"""

import sys

sys.path.insert(0, "/opt/trn_rl_repo")

import numpy as np

import concourse.bacc as bacc
import concourse.bass as bass
import concourse.mybir as mybir
import concourse.tile as tile

dt = mybir.dt
AF = mybir.ActivationFunctionType
AX = mybir.AxisListType

N_CORES = 8
B = 8192
B_LOC = B // N_CORES          # 1024
I = 1024
H = 1024
D = 2 * H                     # 2048
NH = 16
HD = 128
S = 3
C = 7
P = 128
IC = I // P                   # 8 input chunks
HJT = H // P                  # 8 jtiles per gate
KC = D // P                   # 16

_CACHE = {}

FP8_X = False   # GRU input projection via fp8e4m3 DoubleRow matmuls
                # (measured 3.7e-2 rel err on device -- over the 2e-2 gate)
FP8_X_RZ = True  # fp8 DoubleRow x-projection for r/z gates only (their
                 # error is attenuated by the sigmoid slope and gating;
                 # measured 1.15e-2 on device vs the 2e-2 gate)
FP8_QK = True   # Q/K projections via fp8e4m3 DoubleRow

# fp8 weights sit at ~0.02 (q: ~0.0018 after HD^-0.5) -- far below e4m3's
# min normal 2^-7. Pre-scale into the normal range, undo after the matmul.
QK_SCALE = (512.0, 64.0)
QK_SCALE_INV = (1.0 / 512.0, 1.0 / 64.0)


def build_nc(phases="abcd", reps=1, fp8_x=None, fp8_qk=None, fp8_x_rz=None):
    if fp8_x is None:
        fp8_x = FP8_X
    if fp8_qk is None:
        fp8_qk = FP8_QK
    if fp8_x_rz is None:
        fp8_x_rz = FP8_X_RZ
    assert not (fp8_x and fp8_x_rz)
    nc = bacc.Bacc("TRN2", target_bir_lowering=False, debug=False,
                   num_devices=N_CORES, dynamic_dma_scratch_size=8192)

    f32, bf16 = dt.float32, dt.bfloat16
    fp8 = dt.float8e4
    DR = mybir.MatmulPerfMode.DoubleRow
    if fp8_x:
        xt8 = nc.dram_tensor("xt8", [S, P, IC, B_LOC], fp8,
                             kind="ExternalInput")
        wx8 = {d: nc.dram_tensor(f"wx8_{d}", [P, IC // 2, 2, 3 * H], fp8,
                                 kind="ExternalInput") for d in ("f", "b")}
        whh = {d: nc.dram_tensor(f"whh_{d}", [P, HJT, 3 * H], bf16,
                                 kind="ExternalInput") for d in ("f", "b")}
    elif fp8_x_rz:
        xt8 = nc.dram_tensor("xt8", [S, P, IC, B_LOC], fp8,
                             kind="ExternalInput")
        xt = nc.dram_tensor("xt", [S, P, IC, B_LOC], bf16,
                            kind="ExternalInput")
        wx8rz = {d: nc.dram_tensor(f"wx8rz_{d}", [P, IC // 2, 2, 2 * H], fp8,
                                   kind="ExternalInput") for d in ("f", "b")}
        wxn = {d: nc.dram_tensor(f"wxn_{d}", [P, IC, H], bf16,
                                 kind="ExternalInput") for d in ("f", "b")}
        whh = {d: nc.dram_tensor(f"whh_{d}", [P, HJT, 3 * H], bf16,
                                 kind="ExternalInput") for d in ("f", "b")}
    else:
        xt = nc.dram_tensor("xt", [S, P, IC, B_LOC], bf16,
                            kind="ExternalInput")
        wcat = {d: nc.dram_tensor(f"wcat_{d}", [P, KC, 3 * H], bf16,
                                  kind="ExternalInput") for d in ("f", "b")}
    if fp8_qk:
        wqk8 = nc.dram_tensor("wqk8", [NH, P, KC // 2, 2, 2 * P], fp8,
                              kind="ExternalInput")
        wv_d = nc.dram_tensor("wv_d", [NH, P, KC, P], bf16,
                              kind="ExternalInput")
    else:
        wqkv = nc.dram_tensor("wqkv", [NH, P, KC, 3 * P], bf16,
                              kind="ExternalInput")
    wo = nc.dram_tensor("wo", [KC, P, KC, P], bf16, kind="ExternalInput")
    wout = nc.dram_tensor("wout", [P, KC, C], bf16, kind="ExternalInput")
    brz = {d: nc.dram_tensor(f"brz_{d}", [P, 2 * HJT, 1], f32,
                             kind="ExternalInput") for d in ("f", "b")}
    negbz = {d: nc.dram_tensor(f"negbz_{d}", [P, HJT, 1], f32,
                               kind="ExternalInput") for d in ("f", "b")}
    bnih = {d: nc.dram_tensor(f"bnih_{d}", [P, HJT, 1], f32,
                              kind="ExternalInput") for d in ("f", "b")}
    bnhh = {d: nc.dram_tensor(f"bnhh_{d}", [P, HJT, 1], f32,
                              kind="ExternalInput") for d in ("f", "b")}
    oh9 = nc.dram_tensor("oh9", [P, 9, 16], bf16, kind="ExternalInput")
    m1 = nc.dram_tensor("m1", [9, 3], bf16, kind="ExternalInput")
    m2 = nc.dram_tensor("m2", [3, 9], bf16, kind="ExternalInput")
    m3 = nc.dram_tensor("m3", [9, 3], bf16, kind="ExternalInput")
    bc3 = nc.dram_tensor("bc3", [3, 3, P], bf16, kind="ExternalInput")
    bout = nc.dram_tensor("bout", [1, C], f32, kind="ExternalInput")
    o_out = nc.dram_tensor("o_out", [B_LOC, C], f32, kind="ExternalOutput")
    sm_out = nc.dram_tensor("sm_out", [B_LOC, C], f32, kind="ExternalOutput")

    def gru_phase(tc, hs_sb):
        # fp8 GRU weights are host-scaled by 64 out of e4m3's subnormal
        # range (bf16 h-part weights scaled to match); gate activations
        # undo it via their input scale.
        sc = (2.0 ** -6) if (fp8_x or fp8_x_rz) else 1.0
        with (tc.tile_pool(name="ga_c", bufs=1) as cpool,
              tc.tile_pool(name="ga_x", bufs=4) as xpool,
              tc.tile_pool(name="ga_w", bufs=3) as wpool,
              tc.tile_pool(name="ga_g", bufs=2) as gpool,
              tc.tile_pool(name="ga_t", bufs=3) as tpool,
              tc.tile_pool(name="ga_ps", bufs=2, space="PSUM") as pps):
            bias = {}
            for d in ("f", "b"):
                t_brz = cpool.tile([P, 2 * HJT, 1], f32, tag=f"brz{d}")
                nc.sync.dma_start(t_brz[:], brz[d][:])
                t_nbz = cpool.tile([P, HJT, 1], f32, tag=f"nbz{d}")
                nc.sync.dma_start(t_nbz[:], negbz[d][:])
                t_bni = cpool.tile([P, HJT, 1], f32, tag=f"bni{d}")
                nc.sync.dma_start(t_bni[:], bnih[d][:])
                t_bnh = cpool.tile([P, HJT, 1], f32, tag=f"bnh{d}")
                nc.sync.dma_start(t_bnh[:], bnhh[d][:])
                bias[d] = (t_brz, t_nbz, t_bni, t_bnh)

            order = [(0, "f", 0), (0, "b", 2), (1, "f", 1),
                     (1, "b", 1), (2, "f", 2), (2, "b", 0)]
            for step, d, t in order:
                t_brz, t_nbz, t_bni, t_bnh = bias[d]
                first = step == 0
                tp = t - 1 if d == "f" else t + 1
                doff = 0 if d == "f" else HJT
                x_halves = []
                x8_halves = []
                for xh in range(2):
                    hsl = slice(xh * (IC // 2), (xh + 1) * (IC // 2))
                    if fp8_x or fp8_x_rz:
                        xv8 = xpool.tile([P, IC // 2, B_LOC], fp8, tag="x8",
                                         name=f"x8_{step}_{d}_{xh}")
                        nc.sync.dma_start(xv8[:], xt8[t][:, hsl, :])
                        x8_halves.append(xv8)
                    if not fp8_x:
                        xv = xpool.tile([P, IC // 2, B_LOC], bf16, tag="x",
                                        name=f"x_{step}_{d}_{xh}")
                        nc.sync.dma_start(xv[:], xt[t][:, hsl, :])
                        x_halves.append(xv)

                def x_chunk(c):
                    return x_halves[c // (IC // 2)][:, c % (IC // 2), :]

                nkc = IC if first else 2 * IC
                for j in range(HJT):
                    # host pre-permutes weight columns: per j the r/z/n gate
                    # columns are adjacent -> one strided DMA
                    if fp8_x:
                        wx_t = wpool.tile([P, IC // 2, 2, 3 * P], fp8,
                                          tag="wx", name=f"wx_{step}_{d}_{j}")
                        nc.sync.dma_start(
                            wx_t[:], wx8[d][:, :, :, j * 3 * P:(j + 1) * 3 * P])
                    elif fp8_x_rz:
                        wxr_t = wpool.tile([P, IC // 2, 2, 2 * P], fp8,
                                           tag="wxr", name=f"wxr_{step}_{d}_{j}")
                        nc.sync.dma_start(
                            wxr_t[:], wx8rz[d][:, :, :, j * 2 * P:(j + 1) * 2 * P])
                        wxn_t = wpool.tile([P, IC, P], bf16, tag="wxn",
                                           name=f"wxn_{step}_{d}_{j}")
                        nc.sync.dma_start(
                            wxn_t[:], wxn[d][:, :, j * P:(j + 1) * P])
                    else:
                        wj = wpool.tile([P, nkc, 3 * P], bf16, tag="wj",
                                        name=f"wj_{step}_{d}_{j}")
                        nc.sync.dma_start(
                            wj[:], wcat[d][:, :nkc, j * 3 * P:(j + 1) * 3 * P])
                    if (fp8_x or fp8_x_rz) and not first:
                        wh_t = wpool.tile([P, HJT, 3 * P], bf16,
                                          tag="whj",
                                          name=f"whj_{step}_{d}_{j}")
                        nc.sync.dma_start(
                            wh_t[:], whh[d][:, :, j * 3 * P:(j + 1) * 3 * P])
                    for bt in range(2):
                        bs = slice(bt * 512, (bt + 1) * 512)

                        if fp8_x or fp8_x_rz:
                            def mm_xdr(ptile, wtile, g, ncols, last):
                                ncp = IC // 2
                                for cp in range(ncp):
                                    rhs = x8_halves[cp // 2][
                                        :, 2 * (cp % 2):2 * (cp % 2) + 2, bs]
                                    nc.tensor.matmul(
                                        ptile[:],
                                        wtile[:, cp, :, g * P:(g + 1) * P],
                                        rhs, start=(cp == 0),
                                        stop=(cp == ncp - 1 and last),
                                        perf_mode=DR)

                            def mm_acc(ptile, g, with_h):
                                if fp8_x:
                                    mm_xdr(ptile, wx_t, g, 3, not with_h)
                                elif g < 2:
                                    mm_xdr(ptile, wxr_t, g, 2, not with_h)
                                else:
                                    for c in range(IC):
                                        nc.tensor.matmul(
                                            ptile[:], wxn_t[:, c, :],
                                            x_chunk(c)[:, bs],
                                            start=(c == 0),
                                            stop=(c == IC - 1 and not with_h))
                                if with_h:
                                    for c in range(HJT):
                                        nc.tensor.matmul(
                                            ptile[:],
                                            wh_t[:, c, g * P:(g + 1) * P],
                                            hs_sb[:, tp, doff + c, bs],
                                            start=False,
                                            stop=(c == HJT - 1))

                            def mm_h(ptile, g):
                                for c in range(HJT):
                                    nc.tensor.matmul(
                                        ptile[:],
                                        wh_t[:, c, g * P:(g + 1) * P],
                                        hs_sb[:, tp, doff + c, bs],
                                        start=(c == 0), stop=(c == HJT - 1))
                        else:
                            wslice = {0: wj[:, :, 0:P], 1: wj[:, :, P:2 * P],
                                      2: wj[:, :, 2 * P:3 * P]}

                            def rhs_chunk(c):
                                return (x_chunk(c)[:, bs] if c < IC
                                        else hs_sb[:, tp, doff + (c - IC), bs])

                            def mm_acc(ptile, g, with_h):
                                nk = 2 * IC if with_h else IC
                                for c in range(nk):
                                    nc.tensor.matmul(ptile[:],
                                                     wslice[g][:, c, :],
                                                     rhs_chunk(c),
                                                     start=(c == 0),
                                                     stop=(c == nk - 1))

                            def mm_h(ptile, g):
                                for c in range(IC, 2 * IC):
                                    nc.tensor.matmul(ptile[:],
                                                     wslice[g][:, c, :],
                                                     rhs_chunk(c),
                                                     start=(c == IC),
                                                     stop=(c == 2 * IC - 1))

                        pr = pps.tile([P, 512], f32, tag="pr")
                        mm_acc(pr, 0, not first)
                        pz = pps.tile([P, 512], f32, tag="pz")
                        mm_acc(pz, 1, not first)
                        pgi = pps.tile([P, 512], f32, tag="pgi")
                        mm_acc(pgi, 2, False)
                        r_sb = gpool.tile([P, 512], f32, tag="r")
                        nc.scalar.activation(r_sb[:], pr[:], AF.Sigmoid,
                                             bias=t_brz[:, j, :], scale=sc)
                        n_sb = gpool.tile([P, 512], f32, tag="n")
                        if first:
                            zc = gpool.tile([P, 512], f32, tag="z")
                            nc.scalar.activation(zc[:], pz[:], AF.Sigmoid,
                                                 bias=t_nbz[:, j, :],
                                                 scale=-sc)
                            nc.scalar.activation(n_sb[:], pgi[:], AF.Tanh,
                                                 bias=t_bni[:, j, :], scale=sc)
                            nc.vector.tensor_mul(
                                hs_sb[:, t, doff + j, bs], zc[:], n_sb[:])
                        else:
                            z_sb = gpool.tile([P, 512], f32, tag="z")
                            nc.scalar.activation(z_sb[:], pz[:], AF.Sigmoid,
                                                 bias=t_brz[:, HJT + j, :],
                                                 scale=sc)
                            pgh = pps.tile([P, 512], f32, tag="pgh")
                            mm_h(pgh, 2)
                            t1 = tpool.tile([P, 512], f32, tag="tmp")
                            nc.vector.tensor_scalar_add(t1[:], pgh[:],
                                                        t_bnh[:, j, :])
                            t2 = tpool.tile([P, 512], f32, tag="tmp")
                            nc.vector.tensor_mul(t2[:], r_sb[:], t1[:])
                            t3 = tpool.tile([P, 512], f32, tag="tmp")
                            nc.vector.tensor_add(t3[:], pgi[:], t2[:])
                            nc.scalar.activation(n_sb[:], t3[:], AF.Tanh,
                                                 bias=t_bni[:, j, :], scale=sc)
                            t4 = tpool.tile([P, 512], f32, tag="tmp")
                            nc.vector.tensor_sub(
                                t4[:], hs_sb[:, tp, doff + j, bs], n_sb[:])
                            t5 = tpool.tile([P, 512], f32, tag="tmp")
                            nc.vector.tensor_mul(t5[:], z_sb[:], t4[:])
                            nc.vector.tensor_add(
                                hs_sb[:, t, doff + j, bs], t5[:], n_sb[:])

    def attn_combine(tc, h, bh, qh, kh, vh, ctx_all, consts, pools):
        """Per-(head, b-half) logits + softmax + ctx accumulation."""
        oh9_sb, m1_sb, m2_sb, m3_sb, bc3_sb = consts
        prodpool, softpool, tmpool, psl, psw = pools
        bhs = slice(bh * 512, (bh + 1) * 512)
        prod9 = prodpool.tile([P, 9, 512], bf16, tag="prod",
                              name=f"prod_{h}_{bh}")
        for tq in range(S):
            for tk in range(S):
                nc.vector.tensor_mul(prod9[:, 3 * tq + tk, :],
                                     qh[:, tq, bhs], kh[:, tk, bhs])
        # logits are O(1e-3): exp cannot overflow, skip max-sub
        L = psl.tile([16, 512], f32, tag="sm", name=f"L_{h}_{bh}")
        for pair in range(9):
            nc.tensor.matmul(L[:], oh9_sb[:, pair, :], prod9[:, pair, :],
                             start=(pair == 0), stop=(pair == 8))
        E = softpool.tile([9, 512], bf16, tag="E", name=f"E_{h}_{bh}")
        nc.scalar.activation(E[:], L[0:9, :], AF.Exp)
        Es = psl.tile([3, 512], f32, tag="sm", name=f"Es_{h}_{bh}")
        nc.tensor.matmul(Es[:], m1_sb[:], E[:], start=True, stop=True)
        Ri = softpool.tile([3, 512], bf16, tag="Ri", name=f"Ri_{h}_{bh}")
        nc.vector.reciprocal(Ri[:], Es[:])
        Rx = psl.tile([9, 512], f32, tag="sm", name=f"Rx_{h}_{bh}")
        nc.tensor.matmul(Rx[:], m2_sb[:], Ri[:], start=True, stop=True)
        Wn = softpool.tile([9, 512], bf16, tag="Wn", name=f"Wn_{h}_{bh}")
        nc.vector.tensor_mul(Wn[:], E[:], Rx[:])
        Ws = psl.tile([3, 512], f32, tag="sm", name=f"Ws_{h}_{bh}")
        nc.tensor.matmul(Ws[:], m3_sb[:], Wn[:], start=True, stop=True)
        WsS = softpool.tile([3, 512], bf16, tag="WsS", name=f"WsS_{h}_{bh}")
        nc.scalar.copy(WsS[:], Ws[:])
        tms = []
        for tk in range(S):
            Wb = psw.tile([P, 512], f32, tag="wb", name=f"Wb_{h}_{bh}_{tk}")
            nc.tensor.matmul(Wb[:], bc3_sb[:, tk, :], WsS[:],
                             start=True, stop=True)
            tm = tmpool.tile([P, 512], f32, tag="tm",
                             name=f"tm_{h}_{bh}_{tk}")
            nc.vector.tensor_mul(tm[:], vh[:, tk, bhs], Wb[:])
            tms.append(tm)
        nc.vector.tensor_add(tms[0][:], tms[0][:], tms[1][:])
        nc.vector.tensor_add(ctx_all[:, h, bhs], tms[0][:], tms[2][:])

    def load_consts(ohpool):
        oh9_sb = ohpool.tile([P, 9, 16], bf16, tag="oh9")
        nc.sync.dma_start(oh9_sb[:], oh9[:])
        m1_sb = ohpool.tile([9, 3], bf16, tag="m1")
        nc.sync.dma_start(m1_sb[:], m1[:])
        m2_sb = ohpool.tile([3, 9], bf16, tag="m2")
        nc.sync.dma_start(m2_sb[:], m2[:])
        m3_sb = ohpool.tile([9, 3], bf16, tag="m3")
        nc.sync.dma_start(m3_sb[:], m3[:])
        bc3_sb = ohpool.tile([3, 3, P], bf16, tag="bc3")
        nc.sync.dma_start(bc3_sb[:], bc3[:])
        return oh9_sb, m1_sb, m2_sb, m3_sb, bc3_sb

    from contextlib import ExitStack

    with tile.TileContext(nc) as tc, \
         nc.allow_low_precision(reason="bf16/fp8 staging within 2e-2 budget"):
      for _rep in range(reps):
        with tc.tile_pool(name="vdp", bufs=1, space="DRAM") as vdp, \
             ExitStack() as es_right:
            if fp8_qk:
                # left stack: hs + phase-A/B1 pools (freed before B2);
                # right stack: hs8 + ctx (outlive hs)
                es_left = ExitStack()
                hsp = es_left.enter_context(
                    tc.tile_pool(name="hsp", bufs=1))
                hs_sb = hsp.tile([P, S, KC, B_LOC], bf16, tag="hs",
                                 name=f"hs_{_rep}")
                if "a" in phases:
                    gru_phase(tc, hs_sb)

                hs8p = es_right.enter_context(
                    tc.tile_pool(name="hs8p", bufs=1, side="right"))
                ctxp = es_right.enter_context(
                    tc.tile_pool(name="ctxp", bufs=1, side="right"))
                ctx_all = ctxp.tile([P, NH, B_LOC], bf16, tag="ctx",
                                    name=f"ctx_{_rep}")
                if "b" in phases:
                    # B1: hs -> fp8 copy + V projections staged to DRAM
                    hs8 = hs8p.tile([P, S, KC, B_LOC], fp8, tag="hs8",
                                    name=f"hs8_{_rep}")
                    v_dram = vdp.tile([NH, P, S, B_LOC], bf16)
                    with (tc.tile_pool(name="gv_w", bufs=2) as wvpool,
                          tc.tile_pool(name="gv_s", bufs=2) as vspool,
                          tc.tile_pool(name="gv_ps", bufs=2,
                                       space="PSUM") as psv):
                        for t in range(S):
                            for c in range(KC):
                                nc.scalar.copy(hs8[:, t, c, :],
                                               hs_sb[:, t, c, :])
                        for h in range(NH):
                            wvt = wvpool.tile([P, KC, P], bf16, tag="wv",
                                              name=f"wv_{h}")
                            nc.sync.dma_start(wvt[:], wv_d[h])
                            vst = vspool.tile([P, S, B_LOC], bf16,
                                              tag="vst", name=f"vst_{h}")
                            for t in range(S):
                                for bh in range(2):
                                    bhs = slice(bh * 512, (bh + 1) * 512)
                                    pv = psv.tile([P, 512], f32, tag="pv",
                                                  name=f"pv_{h}_{t}_{bh}")
                                    for c in range(KC):
                                        nc.tensor.matmul(
                                            pv[:], wvt[:, c, :],
                                            hs_sb[:, t, c, bhs],
                                            start=(c == 0),
                                            stop=(c == KC - 1))
                                    nc.scalar.copy(vst[:, t, bhs], pv[:])
                            nc.sync.dma_start(v_dram[h], vst[:])
                es_left.close()  # hs (bf16) freed here

                if "b" in phases:
                    # B2: per-head Q/K (fp8 DoubleRow) + attention combine
                    with (tc.tile_pool(name="gb_c", bufs=1) as ohpool,
                          tc.tile_pool(name="gb_w", bufs=2) as whpool,
                          tc.tile_pool(name="gb_qk", bufs=2) as qkpool,
                          tc.tile_pool(name="gb_v", bufs=2) as vpool,
                          tc.tile_pool(name="gb_pr", bufs=2) as prodpool,
                          tc.tile_pool(name="gb_sm", bufs=2) as softpool,
                          tc.tile_pool(name="gb_tm", bufs=3) as tmpool,
                          tc.tile_pool(name="gb_psq", bufs=2,
                                       space="PSUM") as psq,
                          tc.tile_pool(name="gb_psl", bufs=4,
                                       space="PSUM") as psl,
                          tc.tile_pool(name="gb_psw", bufs=2,
                                       space="PSUM") as psw):
                        consts = load_consts(ohpool)
                        pools = (prodpool, softpool, tmpool, psl, psw)
                        for h in range(NH):
                            wqk = whpool.tile([P, KC // 2, 2, 2 * P], fp8,
                                              tag="wqk", name=f"wqk_{h}")
                            nc.sync.dma_start(wqk[:], wqk8[h])
                            vh = vpool.tile([P, S, B_LOC], bf16, tag="v",
                                            name=f"vh_{h}")
                            nc.sync.dma_start(vh[:], v_dram[h])
                            qk = []
                            for g, gn in enumerate(("q", "k")):
                                gt = qkpool.tile([P, S, B_LOC], bf16, tag=gn,
                                                 name=f"{gn}_{h}")
                                qk.append(gt)
                                for t in range(S):
                                    for bh in range(2):
                                        bhs = slice(bh * 512, (bh + 1) * 512)
                                        pq = psq.tile([P, 512], f32, tag="pq",
                                                      name=f"pq_{h}_{g}_{t}_{bh}")
                                        for cp in range(KC // 2):
                                            nc.tensor.matmul(
                                                pq[:],
                                                wqk[:, cp, :, g * P:(g + 1) * P],
                                                hs8[:, t, 2 * cp:2 * cp + 2, bhs],
                                                start=(cp == 0),
                                                stop=(cp == KC // 2 - 1),
                                                perf_mode=DR)
                                        # fp8 weights are pre-scaled out of
                                        # e4m3's subnormal range; undo here
                                        nc.scalar.mul(gt[:, t, bhs], pq[:],
                                                      QK_SCALE_INV[g])
                            for bh in range(2):
                                attn_combine(tc, h, bh, qk[0], qk[1], vh,
                                             ctx_all, consts, pools)
            else:
                hsp = es_right.enter_context(tc.tile_pool(name="hsp", bufs=1))
                hs_sb = hsp.tile([P, S, KC, B_LOC], bf16, tag="hs",
                                 name=f"hs_{_rep}")
                if "a" in phases:
                    gru_phase(tc, hs_sb)
                ctxp = es_right.enter_context(tc.tile_pool(name="ctxp", bufs=1))
                ctx_all = ctxp.tile([P, NH, B_LOC], bf16, tag="ctx",
                                    name=f"ctx_{_rep}")
                if "b" in phases:
                    # bf16 fallback: fused per-head Q/K/V from SBUF hs
                    with (tc.tile_pool(name="gb_c", bufs=1) as ohpool,
                          tc.tile_pool(name="gb_w", bufs=2) as whpool,
                          tc.tile_pool(name="gb_qk", bufs=1) as qkpool,
                          tc.tile_pool(name="gb_v", bufs=2) as vpool,
                          tc.tile_pool(name="gb_pr", bufs=1) as prodpool,
                          tc.tile_pool(name="gb_sm", bufs=2) as softpool,
                          tc.tile_pool(name="gb_tm", bufs=3) as tmpool,
                          tc.tile_pool(name="gb_psq", bufs=2,
                                       space="PSUM") as psq,
                          tc.tile_pool(name="gb_psl", bufs=4,
                                       space="PSUM") as psl,
                          tc.tile_pool(name="gb_psw", bufs=2,
                                       space="PSUM") as psw):
                        consts = load_consts(ohpool)
                        pools = (prodpool, softpool, tmpool, psl, psw)
                        for h in range(NH):
                            wh = whpool.tile([P, KC, 3 * P], bf16, tag="wh",
                                             name=f"wh_{h}")
                            nc.sync.dma_start(wh[:], wqkv[h])
                            qkv = []
                            for g, gn in enumerate(("q", "k", "v")):
                                gpool_ = vpool if gn == "v" else qkpool
                                gt = gpool_.tile([P, S, B_LOC], bf16, tag=gn,
                                                 name=f"{gn}_{h}")
                                qkv.append(gt)
                                for t in (1, 2, 0):
                                    for bh in range(2):
                                        bhs = slice(bh * 512, (bh + 1) * 512)
                                        pq = psq.tile([P, 512], f32, tag="pq",
                                                      name=f"pq_{h}_{g}_{t}_{bh}")
                                        for c in range(KC):
                                            nc.tensor.matmul(
                                                pq[:],
                                                wh[:, c, g * P:(g + 1) * P],
                                                hs_sb[:, t, c, bhs],
                                                start=(c == 0),
                                                stop=(c == KC - 1))
                                        nc.scalar.copy(gt[:, t, bhs], pq[:])
                            for bh in range(2):
                                attn_combine(tc, h, bh, qkv[0], qkv[1], qkv[2],
                                             ctx_all, consts, pools)

            # ---------------- Phase C: Wo projection ----------------
            with tc.tile_pool(name="attp", bufs=1) as attp:
              att_sb = attp.tile([P, KC, B_LOC], bf16, tag="att",
                                 name=f"att_{_rep}")
              if "c" in phases:
                with (tc.tile_pool(name="gc_w", bufs=2) as wopool,
                      tc.tile_pool(name="gc_ps", bufs=2, space="PSUM") as pps):
                    for jt in range(KC):
                        wos = wopool.tile([P, KC, P], bf16, tag="wos",
                                          name=f"wos_{jt}")
                        nc.sync.dma_start(wos[:], wo[jt])
                        for bh in range(2):
                            bhs = slice(bh * 512, (bh + 1) * 512)
                            pw = pps.tile([P, 512], f32, tag="pwo",
                                          name=f"pwo_{jt}_{bh}")
                            for c in range(KC):
                                nc.tensor.matmul(pw[:], wos[:, c, :],
                                                 ctx_all[:, c, bhs],
                                                 start=(c == 0),
                                                 stop=(c == KC - 1))
                            nc.scalar.copy(att_sb[:, jt, bhs], pw[:])

              # ------------- Phase D: classifier + softmax -------------
              if "d" in phases:
                with (tc.tile_pool(name="ge", bufs=2) as epool,
                      tc.tile_pool(name="ge1", bufs=1) as e1pool,
                      tc.tile_pool(name="ge_ps", bufs=2, space="PSUM") as pps2):
                    wout_sb = e1pool.tile([P, KC, C], bf16, tag="wout")
                    nc.sync.dma_start(wout_sb[:], wout[:])
                    bout_sb = e1pool.tile([P, C], f32, tag="bout")
                    nc.sync.dma_start(bout_sb[:], bout[:].to_broadcast([P, C]))
                    for btile in range(HJT):
                        bsl = slice(btile * P, (btile + 1) * P)
                        pf = pps2.tile([P, C], f32, tag="pf")
                        for c in range(KC):
                            nc.tensor.matmul(pf[:], att_sb[:, c, bsl],
                                             wout_sb[:, c, :],
                                             start=(c == 0), stop=(c == KC - 1))
                        o_sb = epool.tile([P, C], f32, tag="osb")
                        nc.vector.tensor_add(o_sb[:], pf[:], bout_sb[:])
                        nc.sync.dma_start(o_out[bsl, :], o_sb[:])
                        mx = epool.tile([P, 1], f32, tag="mx")
                        nc.vector.reduce_max(mx[:], o_sb[:], axis=AX.X)
                        nmx = epool.tile([P, 1], f32, tag="nmx")
                        nc.vector.tensor_scalar_mul(nmx[:], mx[:], -1.0)
                        esb = epool.tile([P, C], f32, tag="esb")
                        nc.scalar.activation(esb[:], o_sb[:], AF.Exp,
                                             bias=nmx[:])
                        ssb = epool.tile([P, 1], f32, tag="ssb")
                        nc.vector.reduce_sum(ssb[:], esb[:], axis=AX.X)
                        rsb = epool.tile([P, 1], f32, tag="rsb")
                        nc.vector.reciprocal(rsb[:], ssb[:])
                        smsb = epool.tile([P, C], f32, tag="smsb")
                        nc.vector.tensor_mul(smsb[:], esb[:],
                                             rsb[:].broadcast_to([P, C]))
                        nc.sync.dma_start(sm_out[bsl, :], smsb[:])

    nc.compile()
    return nc


def _prep_inputs(inputs):
    import ml_dtypes
    f32 = np.float32
    bf = ml_dtypes.bfloat16
    f8 = ml_dtypes.float8_e4m3
    xs = np.stack([np.asarray(inputs["x1"], f32), np.asarray(inputs["x2"], f32),
                   np.asarray(inputs["x3"], f32)])  # (3, B, I)
    shared = {}
    for d in ("f", "b"):
        wih = np.asarray(inputs[f"W_ih_{d}"], f32)
        whh = np.asarray(inputs[f"W_hh_{d}"], f32)
        bih = np.asarray(inputs[f"b_ih_{d}"], f32)
        bhh = np.asarray(inputs[f"b_hh_{d}"], f32)
        wc = np.concatenate([wih.T, whh.T], axis=0)  # (2I, 3H)
        cols = []
        for j in range(HJT):
            for g in range(3):
                cols.append(wc[:, (g * H + j * P):(g * H + (j + 1) * P)])
        wcat = np.concatenate(cols, axis=1)  # (2048, 3072), j-adjacent gates
        shared[f"wcat_{d}"] = np.ascontiguousarray(
            wcat.reshape(KC, P, 3 * H).transpose(1, 0, 2).astype(bf))
        GS = 64.0  # fp8 subnormal-avoidance scale (see QK_SCALE comment)
        if FP8_X:
            shared[f"wx8_{d}"] = np.ascontiguousarray(
                (wcat[:I] * GS).reshape(IC // 2, 2, P, 3 * H)
                .transpose(2, 0, 1, 3).astype(f8))
        if FP8_X_RZ:
            # per-j [r, z] gate columns (first 256 of each 384 block)
            wrz = wcat[:I].reshape(I, HJT, 3 * P)[:, :, :2 * P].reshape(I, -1)
            shared[f"wx8rz_{d}"] = np.ascontiguousarray(
                (wrz * GS).reshape(IC // 2, 2, P, 2 * H)
                .transpose(2, 0, 1, 3).astype(f8))
            wn_ = wcat[:I].reshape(I, HJT, 3 * P)[:, :, 2 * P:].reshape(I, H)
            shared[f"wxn_{d}"] = np.ascontiguousarray(
                (wn_ * GS).reshape(IC, P, H).transpose(1, 0, 2).astype(bf))
        if FP8_X or FP8_X_RZ:
            shared[f"whh_{d}"] = np.ascontiguousarray(
                (wcat[I:] * GS).reshape(HJT, P, 3 * H)
                .transpose(1, 0, 2).astype(bf))
        bsum = bih + bhh
        shared[f"brz_{d}"] = np.ascontiguousarray(
            bsum[:2 * H].reshape(2 * HJT, P).T[:, :, None])
        shared[f"negbz_{d}"] = np.ascontiguousarray(
            (-bsum[H:2 * H]).reshape(HJT, P).T[:, :, None])
        shared[f"bnih_{d}"] = np.ascontiguousarray(
            bih[2 * H:].reshape(HJT, P).T[:, :, None])
        bnhh_s = 64.0 if (FP8_X or FP8_X_RZ) else 1.0
        shared[f"bnhh_{d}"] = np.ascontiguousarray(
            (bhh[2 * H:] * bnhh_s).reshape(HJT, P).T[:, :, None])

    wq_t = (np.asarray(inputs["Wq"], f32).T * (HD ** -0.5))
    wk_t = np.asarray(inputs["Wk"], f32).T
    wv_t = np.asarray(inputs["Wv"], f32).T
    if FP8_QK:
        # [h, p, cp, i2, g*128+o] = Wg.T[(2cp+i2)*128+p, h*128+o]
        qk = np.stack(
            [(w * s).reshape(KC // 2, 2, P, NH, HD).transpose(3, 2, 0, 1, 4)
             for w, s in ((wq_t, QK_SCALE[0]), (wk_t, QK_SCALE[1]))],
            axis=4)  # [NH, P, 8, 2, 2, HD]
        shared["wqk8"] = np.ascontiguousarray(
            qk.reshape(NH, P, KC // 2, 2, 2 * P).astype(f8))
        shared["wv_d"] = np.ascontiguousarray(
            wv_t.reshape(KC, P, NH, HD).transpose(2, 1, 0, 3).astype(bf))
    else:
        qkv = np.stack(
            [w.reshape(KC, P, NH, HD).transpose(2, 1, 0, 3)
             for w in (wq_t, wk_t, wv_t)], axis=3)  # [NH, P, KC, 3, HD]
        shared["wqkv"] = np.ascontiguousarray(
            qkv.reshape(NH, P, KC, 3 * P).astype(bf))

    wo_t = np.asarray(inputs["Wo"], f32).T  # [i, o]
    shared["wo"] = np.ascontiguousarray(
        wo_t.reshape(KC, P, KC, P).transpose(2, 1, 0, 3).astype(bf))

    shared["wout"] = np.ascontiguousarray(
        np.asarray(inputs["W_out"], f32).T.reshape(KC, P, C)
        .transpose(1, 0, 2).astype(bf))
    shared["bout"] = np.ascontiguousarray(
        np.asarray(inputs["b_out"], f32)[None, :])

    oh9 = np.zeros((P, 9, 16), bf)
    for pair in range(9):
        oh9[:, pair, pair] = 1.0
    shared["oh9"] = oh9
    m1 = np.zeros((9, 3), bf)
    m2 = np.zeros((3, 9), bf)
    m3 = np.zeros((9, 3), bf)
    for pr in range(9):
        m1[pr, pr // 3] = 1.0
        m2[pr // 3, pr] = 1.0
        m3[pr, pr % 3] = 1.0
    shared["m1"], shared["m2"], shared["m3"] = m1, m2, m3
    bc3 = np.zeros((3, 3, P), bf)
    for tk in range(3):
        bc3[tk, tk, :] = 1.0
    shared["bc3"] = bc3

    in_maps = []
    for cc in range(N_CORES):
        rows = slice(cc * B_LOC, (cc + 1) * B_LOC)
        m = dict(shared)
        xc = xs[:, rows, :].transpose(0, 2, 1)  # (3, I, B_LOC)
        xtiled = xc.reshape(S, IC, P, B_LOC).transpose(0, 2, 1, 3)
        m["xt"] = np.ascontiguousarray(xtiled.astype(bf))
        if FP8_X or FP8_X_RZ:
            m["xt8"] = np.ascontiguousarray(xtiled.astype(f8))
        in_maps.append(m)
    return in_maps


def _get_nc():
    if "nc" not in _CACHE:
        _CACHE["nc"] = build_nc()
    return _CACHE["nc"]


def kernel(**inputs):
    from concourse.bass_utils import run_bass_kernel_spmd

    nc = _get_nc()
    in_maps = _prep_inputs(inputs)
    res = run_bass_kernel_spmd(nc, in_maps, core_ids=list(range(N_CORES)))
    o = np.concatenate([res.results[c]["o_out"] for c in range(N_CORES)], axis=0)
    sm = np.concatenate([res.results[c]["sm_out"] for c in range(N_CORES)], axis=0)
    return o, sm
